# revision 2
# baseline (speedup 1.0000x reference)
"""Causal multi-head attention (B=2, T=2048, D=1024, NH=16, HD=64) on 8 trn2
NeuronCores.

Sharding: data-parallel over batch (2) x tensor-parallel over head groups (4),
Megatron-style. Core c handles batch c//4, heads 4*(c%4)..4*(c%4)+3: it
computes qkv with the column slice of w_qkv for its heads, full causal
attention for those heads, and the partial output projection with the matching
row slice of w_proj. The host sums the 4 partial projections per batch.

On-device layout is feature-on-partition ("transposed") throughout:
  qk^T [512, T], S^T [k, q] blocks, attention output O^T, final out^T.
The host transposes x on the way in and the partial outputs on the way out.

Matmuls run as float32r (full PE rate, ~tf32-ish rounding, rel err ~1.5e-4 per
matmul). Softmax skips max-subtraction (scores are O(1) by construction), and
the causal mask is applied by zeroing exp(S) on diagonal blocks via gpsimd
affine_select (exp(-1e9) == 0 in the reference, so results match). The softmax
denominator comes free from a ones column appended to V (PV matmul row 64 =
sum_k P). S^T matmuls for the two heads of a pair are row-packed into the same
PE windows via tile_position (contraction is only 64).
"""

import sys

if "/opt/trn_rl_repo" not in sys.path:
    sys.path.insert(0, "/opt/trn_rl_repo")

import numpy as np
import concourse.mybir as mybir
from concourse import bacc
from concourse.tile import TileContext
from concourse import bass_utils

B, T, D = 2, 2048, 1024
NH, HD = 16, 64
HL = 4  # heads per core
N_CORES = 8

KT = D // 128  # 8 contraction tiles over model dim
TCH = T // 512  # 4 q-chunks of 512
TT = T // 128  # 16 t-blocks of 128
KG = 2  # S^T k-blocks per psum group

F32R = mybir.dt.float32r
F32 = mybir.dt.float32


def build_nc():
    nc = bacc.Bacc()
    xT = nc.dram_tensor("xT", [D, T], F32R, kind="ExternalInput")
    wqk = nc.dram_tensor("wqk", [D, 512], F32R, kind="ExternalInput")
    wv = nc.dram_tensor("wv", [D, 256], F32R, kind="ExternalInput")
    wp = nc.dram_tensor("wp", [256, D], F32R, kind="ExternalInput")
    onesc = nc.dram_tensor("onesc", [128, HL], F32R, kind="ExternalInput")
    outT = nc.dram_tensor("outT", [D, T], F32, kind="ExternalOutput")

    with TileContext(nc) as tc:
        with (
            tc.tile_pool(name="persist", bufs=1) as pers,
            tc.tile_pool(name="small", bufs=1) as spool,
        ):
            qkT_sb = [
                pers.tile([128, T], F32R, tag=f"qkT{mt}", name=f"qkT{mt}")
                for mt in range(4)
            ]
            # V1[tt]: [128 t, 4 heads, 65] -- col 64 is the ones column
            V1_sb = [
                pers.tile([128, HL, 65], F32R, tag=f"V1_{tt}", name=f"V1_{tt}")
                for tt in range(TT)
            ]
            AT_sb = [
                pers.tile([128, T], F32R, tag=f"AT{p}", name=f"AT{p}")
                for p in range(2)
            ]
            wp_sb = [
                pers.tile([128, D], F32R, tag=f"wp{ft}", name=f"twp{ft}")
                for ft in range(2)
            ]

            # ---- phase A: qk^T = wqk.T @ x (m-tile order: pair-0 first),
            # ---- phase B: V natural = x @ wv --------------------------------
            with (
                tc.tile_pool(name="qkv_in", bufs=KT) as qin,
                tc.tile_pool(name="psA", bufs=3, space="PSUM") as psa_pool,
                tc.tile_pool(name="psB", bufs=2, space="PSUM") as psb_pool,
            ):
                wqk_sb, wv_sb, xT_sb = [], [], []
                for kt in range(KT):
                    twqk = qin.tile([128, 512], F32R, tag="wqk")
                    nc.sync.dma_start(
                        out=twqk, in_=wqk[kt * 128 : (kt + 1) * 128, :]
                    )
                    wqk_sb.append(twqk)
                    tx = qin.tile([128, T], F32R, tag="xT")
                    for hh in range(2):
                        dma_eng = [nc.sync, nc.scalar][(kt + hh) % 2]
                        dma_eng.dma_start(
                            out=tx[:, hh * 1024 : (hh + 1) * 1024],
                            in_=xT[
                                kt * 128 : (kt + 1) * 128,
                                hh * 1024 : (hh + 1) * 1024,
                            ],
                        )
                    xT_sb.append(tx)
                for kt in range(KT):
                    twv = qin.tile([128, 256], F32R, tag="wv")
                    nc.sync.dma_start(out=twv, in_=wv[kt * 128 : (kt + 1) * 128, :])
                    wv_sb.append(twv)
                for ft in range(2):
                    nc.sync.dma_start(
                        out=wp_sb[ft], in_=wp[ft * 128 : (ft + 1) * 128, :]
                    )
                for tt in range(TT):
                    nc.sync.dma_start(
                        out=V1_sb[tt][:, :, 64:65], in_=onesc[:, :, None]
                    )

                for i, mt in enumerate([0, 2, 1, 3]):
                    for half in range(2):
                        ps = psa_pool.tile(
                            [128, 1024], F32, tag="qk", name=f"qkps{mt}_{half}"
                        )
                        for kt in range(KT):
                            for t2 in range(2):
                                nc.tensor.matmul(
                                    ps[:, t2 * 512 : (t2 + 1) * 512],
                                    wqk_sb[kt][:, mt * 128 : (mt + 1) * 128],
                                    xT_sb[kt][
                                        :,
                                        half * 1024
                                        + t2 * 512 : half * 1024
                                        + (t2 + 1) * 512,
                                    ],
                                    start=(kt == 0),
                                    stop=(kt == KT - 1),
                                )
                        if (2 * i + half) % 2 == 0:
                            nc.vector.tensor_copy(
                                qkT_sb[mt][:, half * 1024 : (half + 1) * 1024], ps
                            )
                        else:
                            nc.scalar.copy(
                                qkT_sb[mt][:, half * 1024 : (half + 1) * 1024], ps
                            )

                for tt in range(TT):
                    psv = psb_pool.tile([128, 256], F32, tag="v", name=f"vps{tt}")
                    for kt in range(KT):
                        nc.tensor.matmul(
                            psv[:, :],
                            xT_sb[kt][:, tt * 128 : (tt + 1) * 128],
                            wv_sb[kt][:, :],
                            start=(kt == 0),
                            stop=(kt == KT - 1),
                        )
                    if tt % 2 == 0:
                        nc.vector.tensor_copy(V1_sb[tt][:, :, 0:64], psv)
                    else:
                        nc.scalar.copy(V1_sb[tt][:, :, 0:64], psv)


            # ---- attention as one global software pipeline over (qc, kb)
            # steps: S/exp lead, affine+PV lag by DEPTH, staging/normalize
            # emitted inline at the lagged position, projection spread in
            # single-tile pieces between S steps -----------------------------
            with (
                tc.tile_pool(name="ptile", bufs=7) as ppool,
                tc.tile_pool(name="stage", bufs=1) as stg,
                tc.tile_pool(name="psS", bufs=2, space="PSUM") as pss_pool,
                tc.tile_pool(name="psO", bufs=4, space="PSUM") as pso_pool,
            ):
                QC_ORDER = [0, 3, 2, 1]
                steps = [(qc, kb) for qc in QC_ORDER for kb in range(4 * qc + 4)]
                DEPTH = 5
                state = {}  # per-qc: oaccs / osb / zall
                proj_pieces = []

                def emit_S(qc, kb):
                    off = 128 * (kb - 4 * qc)
                    lo = max(off, 0)
                    pts = []
                    for p in range(2):
                        qT = qkT_sb[p]
                        kT = qkT_sb[2 + p]
                        psS = pss_pool.tile(
                            [128, 2, 512], F32, tag="s", name=f"s{p}{qc}{kb}"
                        )
                        pt = ppool.tile(
                            [128, 2, 512], F32R, tag="pt", name=f"pt{p}{kb}"
                        )
                        pts.append(pt)
                        for hslot in range(2):
                            nc.tensor.matmul(
                                psS[:, hslot, lo:512],
                                kT[
                                    64 * hslot : 64 * hslot + 64,
                                    kb * 128 : (kb + 1) * 128,
                                ],
                                qT[
                                    64 * hslot : 64 * hslot + 64,
                                    qc * 512 + lo : (qc + 1) * 512,
                                ],
                                start=True,
                                stop=True,
                            )
                        # exp (scale=1/8 fused); diag blocks only live columns
                        nc.scalar.activation(
                            pt[:, :, lo:512],
                            psS[:, :, lo:512],
                            mybir.ActivationFunctionType.Exp,
                            scale=0.125,
                        )
                        if off >= 0:  # diagonal block: causal zeroing
                            # hoisted here (DEPTH steps before the PV matmuls
                            # consume pt) so Pool is off the PV critical path
                            for hslot in range(2):
                                nc.gpsimd.affine_select(
                                    pt[:, hslot, lo:512],
                                    pt[:, hslot, lo:512],
                                    pattern=[[1, 512 - lo]],
                                    compare_op=mybir.AluOpType.is_ge,
                                    fill=0.0,
                                    base=-off + lo,
                                    channel_multiplier=-1,
                                )
                    state[(qc, kb)] = pts

                def emit_PV(qc, kb):
                    nkb = 4 * qc + 4
                    if kb == 0:
                        state[qc] = [
                            pso_pool.tile(
                                [65, 512], F32, tag="o", name=f"o{qc}_{i}"
                            )
                            for i in range(4)
                        ]
                    oaccs = state[qc]
                    off = 128 * (kb - 4 * qc)
                    pts = state.pop((qc, kb))
                    for p in range(2):
                        pt = pts[p]
                        lo = max(off, 0)
                        for hslot in range(2):
                            nc.tensor.matmul(
                                oaccs[2 * p + hslot][:, lo:512],
                                V1_sb[kb][:, 2 * p + hslot, :],
                                pt[:, hslot, lo:512],
                                start=(kb == 0),
                                stop=(kb == nkb - 1),
                            )
                    if kb == nkb - 1:
                        emit_normalize(qc)

                def emit_normalize(qc):
                    oaccs = state.pop(qc)
                    zall = stg.tile([128, 512], F32, tag="z", bufs=2, name=f"z{qc}")
                    osb = [
                        stg.tile(
                            [65, 512], F32, tag=f"osb{i}", bufs=2, name=f"osb{qc}_{i}"
                        )
                        for i in range(4)
                    ]
                    # Z rows first so the reciprocal starts early; then O'
                    # staging (releases the psum accumulators)
                    last = len(state) == 0
                    for i in range(4):
                        if last and i % 2 == 1:
                            nc.scalar.copy(
                                zall[32 * i : 32 * i + 1, :], oaccs[i][64:65, :]
                            )
                        else:
                            nc.vector.tensor_copy(
                                zall[32 * i : 32 * i + 1, :], oaccs[i][64:65, :]
                            )
                    for i in range(4):
                        if last and i % 2 == 1:
                            nc.scalar.copy(osb[i][0:64, :], oaccs[i][0:64, :])
                        else:
                            nc.vector.tensor_copy(osb[i][0:64, :], oaccs[i][0:64, :])
                    rall = stg.tile([128, 512], F32, tag="r", bufs=2, name=f"r{qc}")
                    rscr = stg.tile(
                        [128, 512], F32, tag="rscr", bufs=2, name=f"rscr{qc}"
                    )
                    nc.vector.reciprocal_approx_accurate(rall, zall, rscr)
                    for i in range(4):
                        p, hslot = divmod(i, 2)
                        r0 = stg.tile(
                            [1, 512], F32, tag="r0", bufs=4, name=f"r0{qc}{i}"
                        )
                        nc.vector.tensor_copy(r0, rall[32 * i : 32 * i + 1, :])
                        rb = stg.tile(
                            [64, 512], F32, tag="rb", bufs=4, name=f"rb{qc}{i}"
                        )
                        nc.gpsimd.partition_broadcast(rb, r0)
                        nc.vector.tensor_mul(
                            AT_sb[p][
                                64 * hslot : 64 * hslot + 64,
                                qc * 512 : (qc + 1) * 512,
                            ],
                            osb[i][0:64, :],
                            rb,
                        )
                    for jt2 in range(4):
                        proj_pieces.append((emit_normalize.step + 6, qc, jt2))

                def emit_proj_piece(qc, jt2):
                    psp = pss_pool.tile(
                        [128, 2, 512], F32, tag="s", name=f"pps{qc}{jt2}"
                    )
                    for sub in range(2):
                        for ft in range(2):
                            nc.tensor.matmul(
                                psp[:, sub, :],
                                wp_sb[ft][
                                    :,
                                    (2 * jt2 + sub) * 128 : (2 * jt2 + sub + 1)
                                    * 128,
                                ],
                                AT_sb[ft][:, qc * 512 : (qc + 1) * 512],
                                start=(ft == 0),
                                stop=(ft == 1),
                            )
                    ost = stg.tile(
                        [128, 2, 512], F32, tag="ost", bufs=4, name=f"ost{qc}{jt2}"
                    )
                    nc.vector.tensor_copy(ost[:, 0, :], psp[:, 0, :])
                    nc.scalar.copy(ost[:, 1, :], psp[:, 1, :])
                    ([nc.sync, nc.scalar][jt2 % 2]).dma_start(
                        out=outT[
                            jt2 * 256 : (jt2 + 1) * 256, qc * 512 : (qc + 1) * 512
                        ].rearrange("(a p) q -> p a q", a=2),
                        in_=ost,
                    )

                proj_cool = 0
                for i in range(len(steps) + DEPTH):
                    emit_normalize.step = i
                    if i < len(steps):
                        emit_S(*steps[i])
                        if (
                            proj_pieces
                            and proj_cool <= 0
                            and proj_pieces[0][0] <= i
                        ):
                            _, pqc, pjt2 = proj_pieces.pop(0)
                            emit_proj_piece(pqc, pjt2)
                            proj_cool = 2
                        else:
                            proj_cool -= 1
                    j = i - DEPTH
                    if j >= 0:
                        emit_PV(*steps[j])
                for _, pqc, pjt2 in proj_pieces:
                    emit_proj_piece(pqc, pjt2)

    nc.finalize()
    return nc


_NC_CACHE = None


def _get_nc():
    global _NC_CACHE
    if _NC_CACHE is None:
        _NC_CACHE = build_nc()
    return _NC_CACHE


def make_in_maps(x, w_qkv, w_proj):
    x = np.asarray(x, dtype=np.float32)
    w_qkv = np.asarray(w_qkv, dtype=np.float32)
    w_proj = np.asarray(w_proj, dtype=np.float32)
    ones = np.ones((128, HL), dtype=np.float32)
    in_maps = []
    for c in range(N_CORES):
        b, g = divmod(c, 4)
        cs = 256 * g
        in_maps.append(
            {
                "xT": np.ascontiguousarray(x[b].T),
                "wqk": np.ascontiguousarray(
                    np.concatenate(
                        [w_qkv[:, cs : cs + 256], w_qkv[:, D + cs : D + cs + 256]],
                        axis=1,
                    )
                ),
                "wv": np.ascontiguousarray(w_qkv[:, 2 * D + cs : 2 * D + cs + 256]),
                "wp": np.ascontiguousarray(w_proj[cs : cs + 256, :]),
                "onesc": ones,
            }
        )
    return in_maps


def assemble(results):
    out = np.empty((B, T, D), dtype=np.float32)
    for b in range(B):
        acc = results[4 * b]["outT"].astype(np.float32)
        for g in range(1, 4):
            acc = acc + results[4 * b + g]["outT"]
        out[b] = acc.T
    return out


def kernel(x, w_qkv, w_proj, trace=False):
    nc = _get_nc()
    in_maps = make_in_maps(x, w_qkv, w_proj)
    res = bass_utils.run_bass_kernel_spmd(
        nc, in_maps, core_ids=list(range(N_CORES)), trace=trace
    )
    out = assemble(res.results)
    if trace:
        kernel.last_exec_time_ns = res.exec_time_ns
        kernel.last_result = res
    return out



# revision 11
# speedup vs baseline: 1.2257x; 1.2257x over previous
"""Causal multi-head attention (B=2, T=2048, D=1024, NH=16, HD=64) on 8 trn2
NeuronCores.

Sharding: data-parallel over batch (2) x tensor-parallel over head groups (4),
Megatron-style. Core c handles batch c//4, heads 4*(c%4)..4*(c%4)+3: it
computes qkv with the column slice of w_qkv for its heads, full causal
attention for those heads, and the partial output projection with the matching
row slice of w_proj. The host sums the 4 partial projections per batch.

On-device layout is feature-on-partition ("transposed") throughout:
  qk^T [512, T], S^T [k, q] blocks, attention output O^T, final out^T.
The host transposes x on the way in and the partial outputs on the way out.

Matmuls run as float32r (full PE rate, ~tf32-ish rounding, rel err ~1.5e-4 per
matmul). Softmax skips max-subtraction (scores are O(1) by construction), and
the causal mask is applied by zeroing exp(S) on diagonal blocks via gpsimd
affine_select (exp(-1e9) == 0 in the reference, so results match). The softmax
denominator comes free from a ones column appended to V (PV matmul row 64 =
sum_k P). S^T matmuls for the two heads of a pair are row-packed into the same
PE windows via tile_position (contraction is only 64).
"""

import sys

if "/opt/trn_rl_repo" not in sys.path:
    sys.path.insert(0, "/opt/trn_rl_repo")

import numpy as np
import concourse.mybir as mybir
from concourse import bacc
from concourse.tile import TileContext
from concourse import bass_utils

B, T, D = 2, 2048, 1024
NH, HD = 16, 64
HL = 4  # heads per core
N_CORES = 8

KT = D // 128  # 8 contraction tiles over model dim
TCH = T // 512  # 4 q-chunks of 512
TT = T // 128  # 16 t-blocks of 128
KG = 2  # S^T k-blocks per psum group

F32R = mybir.dt.float32r
F32 = mybir.dt.float32
BF16 = mybir.dt.bfloat16


def build_nc():
    nc = bacc.Bacc()
    xT = nc.dram_tensor("xT", [D, T], BF16, kind="ExternalInput")
    wqk = nc.dram_tensor("wqk", [D, 512], BF16, kind="ExternalInput")
    wv = nc.dram_tensor("wv", [D, 256], BF16, kind="ExternalInput")
    wp = nc.dram_tensor("wp", [256, D], BF16, kind="ExternalInput")
    onesc = nc.dram_tensor("onesc", [128, HL], BF16, kind="ExternalInput")
    tric = nc.dram_tensor("tric", [128, 2, 128], BF16, kind="ExternalInput")
    outT = nc.dram_tensor("outT", [D, T], F32, kind="ExternalOutput")

    with TileContext(nc) as tc:
        with (
            tc.tile_pool(name="persist", bufs=1) as pers,
            tc.tile_pool(name="small", bufs=1) as spool,
        ):
            qkT_sb = [
                pers.tile([128, T], BF16, tag=f"qkT{mt}", name=f"qkT{mt}")
                for mt in range(4)
            ]
            # tri[r, :, j] = 1.0 if r <= j else 0 -- the only part of the
            # causal mask that differs from all-ones is the 128-wide column
            # band at the diagonal, and there it is this same upper-tri
            # pattern for every (qc, kb, head). Built once below; applied as
            # a single DVE multiply per (p, diagonal step).
            tri_sb = pers.tile([128, 2, 128], BF16, tag="tri", name="tri")
            # V1[tt]: [128 t, 4 heads, 65] -- col 64 is the ones column
            V1_sb = [
                pers.tile([128, HL, 65], BF16, tag=f"V1_{tt}", name=f"V1_{tt}")
                for tt in range(TT)
            ]
            AT_sb = [
                pers.tile([128, T], BF16, tag=f"AT{p}", name=f"AT{p}")
                for p in range(2)
            ]
            wp_sb = [
                pers.tile([128, D], BF16, tag=f"wp{ft}", name=f"twp{ft}")
                for ft in range(2)
            ]

            # ---- phase A: qk^T = wqk.T @ x (m-tile order: pair-0 first),
            # ---- phase B: V natural = x @ wv --------------------------------
            with (
                tc.tile_pool(name="qkv_in", bufs=KT) as qin,
                tc.tile_pool(name="psA", bufs=3, space="PSUM") as psa_pool,
                tc.tile_pool(name="psB", bufs=2, space="PSUM") as psb_pool,
            ):
                wqk_sb, wv_sb, xT_sb = [], [], []
                for kt in range(KT):
                    twqk = qin.tile([128, 512], BF16, tag="wqk")
                    nc.sync.dma_start(
                        out=twqk, in_=wqk[kt * 128 : (kt + 1) * 128, :]
                    )
                    wqk_sb.append(twqk)
                    tx = qin.tile([128, T], BF16, tag="xT")
                    for hh in range(2):
                        dma_eng = [nc.sync, nc.scalar][(kt + hh) % 2]
                        dma_eng.dma_start(
                            out=tx[:, hh * 1024 : (hh + 1) * 1024],
                            in_=xT[
                                kt * 128 : (kt + 1) * 128,
                                hh * 1024 : (hh + 1) * 1024,
                            ],
                        )
                    xT_sb.append(tx)
                for kt in range(KT):
                    twv = qin.tile([128, 256], BF16, tag="wv")
                    nc.sync.dma_start(out=twv, in_=wv[kt * 128 : (kt + 1) * 128, :])
                    wv_sb.append(twv)
                for ft in range(2):
                    nc.sync.dma_start(
                        out=wp_sb[ft], in_=wp[ft * 128 : (ft + 1) * 128, :]
                    )
                for tt in range(TT):
                    nc.sync.dma_start(
                        out=V1_sb[tt][:, :, 64:65], in_=onesc[:, :, None]
                    )
                nc.sync.dma_start(out=tri_sb, in_=tric[:, :, :])

                for i, mt in enumerate([0, 2, 1, 3]):
                    for half in range(2):
                        ps = psa_pool.tile(
                            [128, 1024], F32, tag="qk", name=f"qkps{mt}_{half}"
                        )
                        for kt in range(KT):
                            for t2 in range(2):
                                nc.tensor.matmul(
                                    ps[:, t2 * 512 : (t2 + 1) * 512],
                                    wqk_sb[kt][:, mt * 128 : (mt + 1) * 128],
                                    xT_sb[kt][
                                        :,
                                        half * 1024
                                        + t2 * 512 : half * 1024
                                        + (t2 + 1) * 512,
                                    ],
                                    start=(kt == 0),
                                    stop=(kt == KT - 1),
                                )
                        if (2 * i + half) % 2 == 0:
                            nc.vector.tensor_copy(
                                qkT_sb[mt][:, half * 1024 : (half + 1) * 1024], ps
                            )
                        else:
                            nc.scalar.copy(
                                qkT_sb[mt][:, half * 1024 : (half + 1) * 1024], ps
                            )

                for tt in range(TT):
                    psv = psb_pool.tile([128, 256], F32, tag="v", name=f"vps{tt}")
                    for kt in range(KT):
                        nc.tensor.matmul(
                            psv[:, :],
                            xT_sb[kt][:, tt * 128 : (tt + 1) * 128],
                            wv_sb[kt][:, :],
                            start=(kt == 0),
                            stop=(kt == KT - 1),
                        )
                    if tt % 2 == 0:
                        nc.vector.tensor_copy(V1_sb[tt][:, :, 0:64], psv)
                    else:
                        nc.scalar.copy(V1_sb[tt][:, :, 0:64], psv)


            # ---- attention as one global software pipeline over (qc, kb)
            # steps: S/exp lead, affine+PV lag by DEPTH, staging/normalize
            # emitted inline at the lagged position, projection spread in
            # single-tile pieces between S steps -----------------------------
            with (
                tc.tile_pool(name="ptile", bufs=12) as ppool,
                tc.tile_pool(name="stage", bufs=1) as stg,
                tc.tile_pool(name="psS", bufs=2, space="PSUM") as pss_pool,
                tc.tile_pool(name="psO", bufs=4, space="PSUM") as pso_pool,
            ):
                QC_ORDER = [0, 3, 2, 1]
                steps = [(qc, kb) for qc in QC_ORDER for kb in range(4 * qc + 4)]
                DEPTH = 5
                state = {}  # per-qc: oaccs / osb / zall
                proj_pieces = []

                def emit_S(qc, kb):
                    off = 128 * (kb - 4 * qc)
                    lo = max(off, 0)
                    pts = []
                    for p in range(2):
                        qT = qkT_sb[p]
                        kT = qkT_sb[2 + p]
                        psS = pss_pool.tile(
                            [128, 2, 512], F32, tag="s", name=f"s{p}{qc}{kb}"
                        )
                        pt = ppool.tile(
                            [128, 2, 512], BF16, tag="pt", name=f"pt{p}{kb}"
                        )
                        pts.append(pt)
                        for hslot in range(2):
                            nc.tensor.matmul(
                                psS[:, hslot, lo:512],
                                kT[
                                    64 * hslot : 64 * hslot + 64,
                                    kb * 128 : (kb + 1) * 128,
                                ],
                                qT[
                                    64 * hslot : 64 * hslot + 64,
                                    qc * 512 + lo : (qc + 1) * 512,
                                ],
                                start=True,
                                stop=True,
                            )
                        # exp (scale=1/8 fused); diag blocks only live columns
                        nc.scalar.activation(
                            pt[:, :, lo:512],
                            psS[:, :, lo:512],
                            mybir.ActivationFunctionType.Exp,
                            scale=0.125,
                        )
                        if off >= 0:  # diagonal block: causal zeroing
                            # only the 128-col band at the diagonal needs
                            # masking (columns beyond it are fully live);
                            # done DEPTH steps before PV consumes pt, off
                            # the PV critical path
                            nc.vector.tensor_mul(
                                pt[:, :, lo : lo + 128],
                                pt[:, :, lo : lo + 128],
                                tri_sb,
                            )
                    state[(qc, kb)] = pts

                def emit_PV(qc, kb):
                    nkb = 4 * qc + 4
                    if kb == 0:
                        state[qc] = [
                            pso_pool.tile(
                                [65, 512], F32, tag="o", name=f"o{qc}_{i}"
                            )
                            for i in range(4)
                        ]
                    oaccs = state[qc]
                    off = 128 * (kb - 4 * qc)
                    pts = state.pop((qc, kb))
                    for p in range(2):
                        pt = pts[p]
                        lo = max(off, 0)
                        for hslot in range(2):
                            nc.tensor.matmul(
                                oaccs[2 * p + hslot][:, lo:512],
                                V1_sb[kb][:, 2 * p + hslot, :],
                                pt[:, hslot, lo:512],
                                start=(kb == 0),
                                stop=(kb == nkb - 1),
                            )
                    if kb == nkb - 1:
                        emit_normalize(qc)

                def emit_normalize(qc):
                    oaccs = state.pop(qc)
                    zall = stg.tile([128, 512], F32, tag="z", bufs=2, name=f"z{qc}")
                    osb = [
                        stg.tile(
                            [65, 512], F32, tag=f"osb{i}", bufs=2, name=f"osb{qc}_{i}"
                        )
                        for i in range(4)
                    ]
                    # Z rows first so the reciprocal starts early; then O'
                    # staging (releases the psum accumulators)
                    last = len(state) == 0
                    for i in range(4):
                        if last and i % 2 == 1:
                            nc.scalar.copy(
                                zall[32 * i : 32 * i + 1, :], oaccs[i][64:65, :]
                            )
                        else:
                            nc.vector.tensor_copy(
                                zall[32 * i : 32 * i + 1, :], oaccs[i][64:65, :]
                            )
                    for i in range(4):
                        if last and i % 2 == 1:
                            nc.scalar.copy(osb[i][0:64, :], oaccs[i][0:64, :])
                        else:
                            nc.vector.tensor_copy(osb[i][0:64, :], oaccs[i][0:64, :])
                    rall = stg.tile([128, 512], F32, tag="r", bufs=2, name=f"r{qc}")
                    rscr = stg.tile(
                        [128, 512], F32, tag="rscr", bufs=2, name=f"rscr{qc}"
                    )
                    nc.vector.reciprocal_approx_accurate(rall, zall, rscr)
                    for i in range(4):
                        p, hslot = divmod(i, 2)
                        r0 = stg.tile(
                            [1, 512], F32, tag="r0", bufs=4, name=f"r0{qc}{i}"
                        )
                        nc.vector.tensor_copy(r0, rall[32 * i : 32 * i + 1, :])
                        rb = stg.tile(
                            [64, 512], F32, tag="rb", bufs=4, name=f"rb{qc}{i}"
                        )
                        nc.gpsimd.partition_broadcast(rb, r0)
                        nc.vector.tensor_mul(
                            AT_sb[p][
                                64 * hslot : 64 * hslot + 64,
                                qc * 512 : (qc + 1) * 512,
                            ],
                            osb[i][0:64, :],
                            rb,
                        )
                    for jt2 in range(4):
                        proj_pieces.append((emit_normalize.step + 6, qc, jt2))

                def emit_proj_piece(qc, jt2):
                    psp = pss_pool.tile(
                        [128, 2, 512], F32, tag="s", name=f"pps{qc}{jt2}"
                    )
                    for sub in range(2):
                        for ft in range(2):
                            nc.tensor.matmul(
                                psp[:, sub, :],
                                wp_sb[ft][
                                    :,
                                    (2 * jt2 + sub) * 128 : (2 * jt2 + sub + 1)
                                    * 128,
                                ],
                                AT_sb[ft][:, qc * 512 : (qc + 1) * 512],
                                start=(ft == 0),
                                stop=(ft == 1),
                            )
                    ost = stg.tile(
                        [128, 2, 512], F32, tag="ost", bufs=4, name=f"ost{qc}{jt2}"
                    )
                    nc.vector.tensor_copy(ost[:, 0, :], psp[:, 0, :])
                    nc.scalar.copy(ost[:, 1, :], psp[:, 1, :])
                    ([nc.sync, nc.scalar][jt2 % 2]).dma_start(
                        out=outT[
                            jt2 * 256 : (jt2 + 1) * 256, qc * 512 : (qc + 1) * 512
                        ].rearrange("(a p) q -> p a q", a=2),
                        in_=ost,
                    )

                proj_cool = 0
                for i in range(len(steps) + DEPTH):
                    emit_normalize.step = i
                    if i < len(steps):
                        emit_S(*steps[i])
                        if (
                            proj_pieces
                            and proj_cool <= 0
                            and proj_pieces[0][0] <= i
                        ):
                            _, pqc, pjt2 = proj_pieces.pop(0)
                            emit_proj_piece(pqc, pjt2)
                            proj_cool = 2
                        else:
                            proj_cool -= 1
                    j = i - DEPTH
                    if j >= 0:
                        emit_PV(*steps[j])
                for _, pqc, pjt2 in proj_pieces:
                    emit_proj_piece(pqc, pjt2)

    nc.finalize()
    return nc


_NC_CACHE = None


def _get_nc():
    global _NC_CACHE
    if _NC_CACHE is None:
        _NC_CACHE = build_nc()
    return _NC_CACHE


def make_in_maps(x, w_qkv, w_proj):
    import ml_dtypes

    bf16 = ml_dtypes.bfloat16
    x = np.asarray(x, dtype=np.float32).astype(bf16)
    w_qkv = np.asarray(w_qkv, dtype=np.float32).astype(bf16)
    w_proj = np.asarray(w_proj, dtype=np.float32).astype(bf16)
    ones = np.ones((128, HL), dtype=bf16)
    tri = np.ascontiguousarray(
        np.broadcast_to(
            np.triu(np.ones((128, 128), dtype=np.float32))[:, None, :],
            (128, 2, 128),
        )
    ).astype(bf16)
    in_maps = []
    for c in range(N_CORES):
        b, g = divmod(c, 4)
        cs = 256 * g
        in_maps.append(
            {
                "xT": np.ascontiguousarray(x[b].T),
                "wqk": np.ascontiguousarray(
                    np.concatenate(
                        [w_qkv[:, cs : cs + 256], w_qkv[:, D + cs : D + cs + 256]],
                        axis=1,
                    )
                ),
                "wv": np.ascontiguousarray(w_qkv[:, 2 * D + cs : 2 * D + cs + 256]),
                "wp": np.ascontiguousarray(w_proj[cs : cs + 256, :]),
                "onesc": ones,
                "tric": tri,
            }
        )
    return in_maps


def assemble(results):
    out = np.empty((B, T, D), dtype=np.float32)
    for b in range(B):
        acc = results[4 * b]["outT"].astype(np.float32)
        for g in range(1, 4):
            acc = acc + results[4 * b + g]["outT"]
        out[b] = acc.T
    return out


def kernel(x, w_qkv, w_proj, trace=False):
    nc = _get_nc()
    in_maps = make_in_maps(x, w_qkv, w_proj)
    res = bass_utils.run_bass_kernel_spmd(
        nc, in_maps, core_ids=list(range(N_CORES)), trace=trace
    )
    out = assemble(res.results)
    if trace:
        kernel.last_exec_time_ns = res.exec_time_ns
        kernel.last_result = res
    return out

